# revision 6
# baseline (speedup 1.0000x reference)
"""GATv2 edge-softmax kernel for 8 Trainium2 cores — V2 (scan segment-sum).

Sharding: core c owns src nodes [c*12512, (c+1)*12512) and ALL edges whose
src falls there, so every segment (softmax row) lives wholly on one core:
no cross-device reduction is needed at all.

Within a core, edges are sorted by src and packed row-major into [128, TC]
with runs never crossing a row boundary (host pads). The segment sums are
then per-row run sums, computed with a log-step segmented suffix-scan +
forward broadcast on the Vector engine — no scatter DMAs.

Per 128-edge column: two indirect-DMA row gathers from the projected-node
table pcat[n]=[p1[n]+b | p2[n]] (src -> cols 0:64; dst -> cols 64:128 with
CCE-add) produce y = p1[src]+p2[dst]+b in SBUF. scores = a . LeakyReLU(y);
attn = exp(scores) * 1/run_total, written back and unpermuted on the host.
"""
import sys
sys.path.insert(0, '/opt/trn_rl_repo')
import numpy as np
import concourse.bass as bass
import concourse.bacc as bacc
import concourse.mybir as mybir
import concourse.tile as tile
from concourse import bass_utils
from concourse.masks import make_identity

P = 128
D = 64
NSLOPE = 0.2

FULL = dict(
    n_nodes=100000,
    n_pad=100096,      # 782 * 128 = 8 * 12512
    e=1600000,
    n_cores=8,
    k=16,              # edge columns per compute tile
    ec=212992,         # 128 * 1664 slots per core (104 tiles of 2048)
    max_deg=128,       # scan shift bound (host asserts true max < this)
)

_cache = {}


def build(cfg):
    n_pad, n_cores, K, EC = cfg["n_pad"], cfg["n_cores"], cfg["k"], cfg["ec"]
    TC = EC // P
    NBT = TC // K
    assert TC % K == 0 and n_pad % P == 0

    nc = bacc.Bacc("TRN2", target_bir_lowering=False, debug=False,
                   num_devices=n_cores)
    f32, i32 = mybir.dt.float32, mybir.dt.int32
    nodes = nc.dram_tensor("nodes", [n_pad, D], f32, kind="ExternalInput").ap()
    wmat = nc.dram_tensor("wmat", [D, 2 * D], f32, kind="ExternalInput").ap()
    biast = nc.dram_tensor("biast", [P, 2 * D], f32, kind="ExternalInput").ap()
    abig = nc.dram_tensor("abig", [P, K * D], f32, kind="ExternalInput").ap()
    srci = nc.dram_tensor("srci", [P, TC], i32, kind="ExternalInput").ap()
    dsti = nc.dram_tensor("dsti", [P, TC], i32, kind="ExternalInput").ap()
    attn = nc.dram_tensor("attn", [P, TC], f32, kind="ExternalOutput").ap()

    # +P pad rows: pad edges point at row n_pad (garbage, masked out later)
    pcat = nc.dram_tensor("pcat", [n_pad + P, 2 * D], f32, kind="Internal").ap()

    with tile.TileContext(nc) as tc:
        with tc.tile_pool(name="persist", bufs=1) as pp, \
             tc.tile_pool(name="sb", bufs=4) as pool, \
             tc.tile_pool(name="scan", bufs=2) as scp, \
             tc.tile_pool(name="ps", bufs=4, space="PSUM") as psp:

            # ---------- setup ----------
            ident = pp.tile([P, P], f32)
            make_identity(nc, ident[:])
            wm_sb = pp.tile([D, 2 * D], f32)
            nc.sync.dma_start(wm_sb[:], wmat[:])
            bias_t = pp.tile([P, 2 * D], f32)
            nc.sync.dma_start(bias_t[:], biast[:])
            a_big = pp.tile([P, K * D], f32)
            nc.sync.dma_start(a_big[:], abig[:])

            src_all = pp.tile([P, TC], i32)
            dst_all = pp.tile([P, TC], i32)
            exp_all = pp.tile([P, TC], f32)
            nc.sync.dma_start(src_all[:], srci[:])
            nc.sync.dma_start(dst_all[:], dsti[:])

            # ---------- phase A: pcat[n] = [p1+b | p2] ----------
            for t in range(n_pad // P):
                nt = pool.tile([P, D], f32, tag="nt")
                nc.sync.dma_start(nt[:], nodes[t * P:(t + 1) * P, :])
                ntT_ps = psp.tile([D, P], f32, tag="ntT")
                nc.tensor.transpose(out=ntT_ps[:], in_=nt[:],
                                    identity=ident[:])
                ntT = pool.tile([D, P], f32, tag="ntTs")
                nc.vector.tensor_copy(out=ntT[:], in_=ntT_ps[:])
                pc_ps = psp.tile([P, 2 * D], f32, tag="pc")
                nc.tensor.matmul(pc_ps[:], lhsT=ntT[:], rhs=wm_sb[:],
                                 start=True, stop=True)
                pc = pool.tile([P, 2 * D], f32, tag="pcs")
                nc.vector.tensor_add(out=pc[:], in0=pc_ps[:], in1=bias_t[:])
                nc.sync.dma_start(pcat[t * P:(t + 1) * P, :], pc[:])

            # zero the pad rows so pad-edge gathers stay finite
            zpad = pool.tile([P, 2 * D], f32, tag="pcs")
            nc.vector.memset(zpad[:], 0.0)
            nc.sync.dma_start(pcat[n_pad:n_pad + P, :], zpad[:])

            # ---------- phase B: scores -> exp_all ----------
            for t in range(NBT):
                cols = slice(t * K, (t + 1) * K)
                g = pool.tile([P, K * D], f32, tag="g")
                for k in range(K):
                    c = t * K + k
                    nc.gpsimd.indirect_dma_start(
                        out=g[:, k * D:(k + 1) * D], out_offset=None,
                        in_=pcat[:],
                        in_offset=bass.IndirectOffsetOnAxis(
                            ap=src_all[:, c:c + 1], axis=0))
                for k in range(K):
                    c = t * K + k
                    nc.gpsimd.indirect_dma_start(
                        out=g[:, k * D:(k + 1) * D], out_offset=None,
                        in_=pcat[:],
                        in_offset=bass.IndirectOffsetOnAxis(
                            ap=dst_all[:, c:c + 1], axis=0),
                        element_offset=D,
                        compute_op=mybir.AluOpType.add)
                act = pool.tile([P, K * D], f32, tag="act")
                nc.vector.tensor_scalar_mul(act[:], g[:], NSLOPE)
                nc.vector.tensor_tensor(out=act[:], in0=act[:], in1=g[:],
                                        op=mybir.AluOpType.max)
                w = pool.tile([P, K * D], f32, tag="w")
                nc.vector.tensor_mul(
                    out=w[:].rearrange("p (k d) -> p k d", d=D),
                    in0=act[:].rearrange("p (k d) -> p k d", d=D),
                    in1=a_big[:].rearrange("p (k d) -> p k d", d=D))
                sc = pool.tile([P, K], f32, tag="sc")
                nc.vector.tensor_reduce(
                    out=sc[:], in_=w[:].rearrange("p (k d) -> p k d", d=D),
                    axis=mybir.AxisListType.X, op=mybir.AluOpType.add)
                nc.scalar.activation(exp_all[:, cols], sc[:],
                                     mybir.ActivationFunctionType.Exp)

            # zero pad-slot exp (pad edges have src == n_pad)
            padmask = pp.tile([P, TC], f32)
            nc.vector.tensor_scalar(out=padmask[:], in0=src_all[:],
                                    scalar1=n_pad, scalar2=None,
                                    op0=mybir.AluOpType.is_lt)
            nc.vector.tensor_mul(out=exp_all[:], in0=exp_all[:],
                                 in1=padmask[:])

            # ---------- phase C: segmented run-sum scan ----------
            # suffix sums within equal-src runs (runs never cross rows)
            tot = pp.tile([P, TC], f32)
            nc.vector.tensor_copy(out=tot[:], in_=exp_all[:])
            s = 1
            while s < cfg["max_deg"] and s < TC:
                m = scp.tile([P, TC], f32, tag="m")
                nc.vector.tensor_tensor(
                    out=m[:, 0:TC - s], in0=src_all[:, s:TC],
                    in1=src_all[:, 0:TC - s], op=mybir.AluOpType.is_equal)
                tmp = scp.tile([P, TC], f32, tag="tmp")
                nc.vector.tensor_mul(out=tmp[:, 0:TC - s],
                                     in0=tot[:, s:TC], in1=m[:, 0:TC - s])
                nc.vector.tensor_add(out=tot[:, 0:TC - s],
                                     in0=tot[:, 0:TC - s],
                                     in1=tmp[:, 0:TC - s])
                s *= 2
            # forward-broadcast run totals (leftmost slot holds the total)
            vcur = tot
            s = 1
            while s < cfg["max_deg"] and s < TC:
                m = scp.tile([P, TC], mybir.dt.uint8, tag="m8")
                nc.vector.tensor_tensor(
                    out=m[:, s:TC], in0=src_all[:, s:TC],
                    in1=src_all[:, 0:TC - s], op=mybir.AluOpType.is_equal)
                vnext = scp.tile([P, TC], f32, tag="v")
                nc.vector.tensor_copy(out=vnext[:, 0:s], in_=vcur[:, 0:s])
                nc.vector.select(out=vnext[:, s:TC], mask=m[:, s:TC],
                                 on_true=vcur[:, 0:TC - s],
                                 on_false=vcur[:, s:TC])
                vcur = vnext
                s *= 2

            rs = pp.tile([P, TC], f32)
            nc.vector.tensor_scalar_add(vcur[:], vcur[:], 1e-30)
            nc.vector.reciprocal(rs[:], vcur[:])
            nc.vector.tensor_mul(out=exp_all[:], in0=exp_all[:], in1=rs[:])
            nc.sync.dma_start(attn[:], exp_all[:])
    nc.compile()
    return nc


def layout_core(src_c, dst_c, n_nodes, EC, TC):
    """Sort by src; pack row-major into [128, TC] so no run crosses a row.
    Returns (srci, dsti, slot_of_edge)."""
    order = np.argsort(src_c, kind="stable")
    ss, dd = src_c[order], dst_c[order]
    n = len(ss)
    starts = np.flatnonzero(np.r_[True, ss[1:] != ss[:-1]])
    lengths = np.diff(np.r_[starts, n])
    slots = np.empty(n, dtype=np.int64)
    row, pos = 0, 0
    for st, L in zip(starts, lengths):
        if pos + L > TC:
            row += 1
            pos = 0
        slots[st:st + L] = row * TC + pos + np.arange(L)
        pos += L
    assert row < P, "edges did not fit the padded layout"
    pad_src = np.int32(FULL["n_pad"])
    srci = np.full(EC, pad_src, dtype=np.int32)
    dsti = np.zeros(EC, dtype=np.int32)
    srci[slots] = ss
    dsti[slots] = dd
    slot_of_edge = np.empty(n, dtype=np.int64)
    slot_of_edge[order] = slots
    return srci.reshape(P, TC), dsti.reshape(P, TC), slot_of_edge


def prep_inputs(cfg, nodes, src, dst, W_w, W_b, a_w):
    n_nodes, n_pad, n_cores = cfg["n_nodes"], cfg["n_pad"], cfg["n_cores"]
    EC = cfg["ec"]
    TC = EC // P
    nodes_pad = np.zeros((n_pad, D), dtype=np.float32)
    nodes_pad[:n_nodes] = nodes
    wmat = np.concatenate([W_w[:, :D].T, W_w[:, D:].T], axis=1).copy()
    K = cfg["k"]
    biast = np.concatenate(
        [np.tile(W_b.reshape(1, D), (P, 1)),
         np.zeros((P, D), np.float32)], axis=1).astype(np.float32)
    abig = np.tile(a_w.reshape(1, D), (P, K)).astype(np.float32)

    nodes_per_core = n_pad // n_cores
    core_of_edge = src // nodes_per_core
    in_maps, edge_ids, slot_maps = [], [], []
    for c in range(n_cores):
        eids = np.flatnonzero(core_of_edge == c)
        srci, dsti, slot_of_edge = layout_core(
            src[eids], dst[eids], n_nodes, EC, TC)
        # degree bound for the scan
        in_maps.append({
            "nodes": nodes_pad, "wmat": wmat, "biast": biast, "abig": abig,
            "srci": srci, "dsti": dsti,
        })
        edge_ids.append(eids)
        slot_maps.append(slot_of_edge)
    return in_maps, edge_ids, slot_maps


def unshard(cfg, results, edge_ids, slot_maps):
    out = np.empty(cfg["e"], dtype=np.float32)
    for c in range(cfg["n_cores"]):
        a = results[c]["attn"].reshape(cfg["ec"])
        out[edge_ids[c]] = a[slot_maps[c]]
    return out


def kernel(nodes, src, dst, W_w, W_b, a_w):
    cfg = FULL
    nodes = np.asarray(nodes, dtype=np.float32)
    src = np.asarray(src, dtype=np.int32)
    dst = np.asarray(dst, dtype=np.int32)
    W_w = np.asarray(W_w, dtype=np.float32)
    W_b = np.asarray(W_b, dtype=np.float32)
    a_w = np.asarray(a_w, dtype=np.float32)

    if "nc" not in _cache:
        _cache["nc"] = build(cfg)
    nc = _cache["nc"]
    in_maps, edge_ids, slot_maps = prep_inputs(
        cfg, nodes, src, dst, W_w, W_b, a_w)
    res = bass_utils.run_bass_kernel_spmd(
        nc, in_maps, core_ids=list(range(cfg["n_cores"])))
    return unshard(cfg, res.results, edge_ids, slot_maps)
